# revision 5
# baseline (speedup 1.0000x reference)
"""Trainium2 Bass kernel for modulated (StyleGAN2-style) 3x3 conv, groups=batch.

Full-input contract: kernel(**inputs) takes the unsharded numpy inputs and
returns the full (16, 512, 64, 64) fp32 output. Internally the batch is
sharded 2-per-core across 8 NeuronCores (data parallel); weight/mod params
are replicated (weights shipped pre-transposed as bf16; the cast is
bit-identical to an on-device round-to-nearest cast).

Math (matching the jax reference):
    s      = style @ mod_w.T + mod_b                      # (B, IC)
    wmod   = SCALE * w * s[:,None,:,None,None]
    demod  = rsqrt(sum(wmod^2, (ic,kh,kw)) + 1e-8)        # (B, OC)
    out    = conv(x, wmod*demod, groups=batch)

Device decomposition per core (2 samples):
    conv(x, w*s) == conv(x*s, w)   -> fold s into the x fp32->bf16 pad/cast
    demod*SCALE  == rsqrt(ss + eps*IC*K*K) with ss = sum_ic WS[oc,ic]*s2[b,ic]
    conv: 1D Winograd F(2,3) along y, direct 3-tap along x:
      V0 = E[t]-E[t+1]  V1 = O[t]+E[t+1]  V2 = E[t+1]-O[t]  V3 = O[t]-O[t+1]
    where E/O are the even/odd rows of the zero-padded scaled image, stored
    deinterleaved so every V combo is a flat contiguous DVE op (2x mode).
      U0 = w[ky=0], U1' = w0+w1+w2, U2' = w0-w1+w2, U3 = w[ky=2]
    (A^T's 0.5 on the U1/U2 terms is folded into the demod scalars.)
      M_k[oc, t, x] = sum_{ic,kx} U_k[ic,oc] * V_k[ic, t, x+kx]   (PE, PSUM)
      out[2t]   = (M0 + .5*M1' + .5*M2') * demod
      out[2t+1] = (.5*M1' - .5*M2' - M3) * demod
    48 matmuls per (band, oc-chunk) vs 72 direct: 1.5x less PE work.
"""

import sys

for _p in ("/opt/trn_rl_repo",):
    if _p not in sys.path:
        sys.path.append(_p)

import ml_dtypes
import numpy as np

import concourse.bass as bass
import concourse.tile as tile
from concourse import mybir
from concourse.bass_utils import run_bass_kernel_spmd

# ---------------------------------------------------------------------------
# Workaround for this container's walrus build: an instruction can carry only
# one semaphore wait (two for EventSemaphore), but Tile emits up to two per
# instruction (and the exit drain gets one per logical processor), which
# walrus rejects with "Too many sync wait commands". Fix at the BIR-JSON
# level: move excess waits onto NoOp carrier instructions inserted directly
# before the offender on the same engine — semantically identical (all waits
# still satisfied before the instruction executes, per-engine order kept).
# ---------------------------------------------------------------------------
import json as _json

_SPLIT_OK_ENGINES = {"PE", "DVE", "Activation", "Pool", "SP"}
_orig_to_json_bytes = bass.Bass.to_json_bytes


def _to_json_bytes_split_waits(self):
    raw = _orig_to_json_bytes(self)
    m = _json.loads(raw)
    changed = False
    for fn in m.get("functions", []):
        for bb in fn.get("blocks", []):
            insts = bb.get("instructions", [])
            new_insts = []
            for inst in insts:
                si = inst.get("sync_info")
                waits = (si or {}).get("on_wait") or []
                op = inst.get("opcode", "")
                limit = 2 if op == "EventSemaphore" else 1
                if len(waits) > limit:
                    eng = inst.get("engine")
                    assert eng in _SPLIT_OK_ENGINES, (
                        f"instruction {inst.get('name')} on engine {eng} has "
                        f"{len(waits)} waits; carrier NoOp not known-safe there"
                    )
                    changed = True
                    keep = waits[-limit:]
                    for i, w in enumerate(waits[:-limit]):
                        new_insts.append(
                            {
                                "debug": inst.get("debug", 0),
                                "engine": eng,
                                "ins": [],
                                "name": f"{inst['name']}.w{i}",
                                "opcode": "NoOp",
                                "outs": [],
                                "sync_info": {"on_wait": [w], "on_update": []},
                            }
                        )
                    si["on_wait"] = keep
                new_insts.append(inst)
            bb["instructions"] = new_insts
    if not changed:
        return raw
    return _json.dumps(m).encode()


bass.Bass.to_json_bytes = _to_json_bytes_split_waits

# ---------------------------------------------------------------------------
# Problem constants (hardcoded per spec)
# ---------------------------------------------------------------------------
B, IC, OC, H, W, KS, SD = 16, 512, 512, 64, 64, 3, 512
NCORES = 8
BPC = B // NCORES          # samples per core
P = 128
NIC = IC // P              # 4 ic chunks
NOC = OC // P              # 4 oc chunks
KK = KS * KS               # 9
PWX = W + 4                # 68 padded row width (4B-aligned rows)
EROWS = H // 2 + 1         # 33 rows in each of the E/O parity planes
TT = H // 2                # 32 winograd row-pair tiles
TB = 8                     # tiles per band/group (16 output rows)
NBK = TT // TB             # 4 bands per sample
NFREE = TB * W             # 512 matmul free dim
VR = 2                     # V ring depth (bands in flight)
RB = 8                     # x staging rows per band
NXB = H // RB              # 8 x-staging bands
# rsqrt(SCALE^2*ss + 1e-8) * SCALE == rsqrt(ss + 1e-8*IC*K*K)
EPS_FOLDED = 1e-8 * IC * KS * KS

F32 = mybir.dt.float32
BF16 = mybir.dt.bfloat16
_ALU = mybir.AluOpType


def build_nc():
    nc = bass.Bass()
    xs = nc.dram_tensor("xs", [BPC, IC, H, W], F32, kind="ExternalInput")
    stT = nc.dram_tensor("stT", [SD, BPC], F32, kind="ExternalInput")
    wTb = nc.dram_tensor("wTb", [IC, KK, OC], BF16, kind="ExternalInput")
    mwT = nc.dram_tensor("mwT", [SD, IC], F32, kind="ExternalInput")
    mb = nc.dram_tensor("mb", [IC], F32, kind="ExternalInput")
    out = nc.dram_tensor("out", [BPC, OC, H, W], F32, kind="ExternalOutput")

    with tile.TileContext(nc) as tc:
        with (
            tc.tile_pool(name="singles", bufs=1) as singles,
            tc.tile_pool(name="sqp", bufs=3) as sqp,
            tc.tile_pool(name="up", bufs=2) as up,
            tc.tile_pool(name="xstage", bufs=4) as xstage,
            tc.tile_pool(name="otmp", bufs=4) as otmp,
            tc.tile_pool(name="outp", bufs=4) as outp,
            tc.tile_pool(name="psum", bufs=8, space="PSUM") as psum,
        ):
            # ---- constants ------------------------------------------------
            mwT_sb = singles.tile([P, SD // P, IC], F32)
            mwT_v = mwT.rearrange("(ko ki) i -> ki ko i", ki=P)
            for k in range(SD // P):
                nc.sync.dma_start(mwT_sb[:, k], mwT_v[:, k])
            stT_sb = singles.tile([P, SD // P, BPC], F32)
            nc.sync.dma_start(stT_sb, stT.rearrange("(ko ki) b -> ki ko b", ki=P))
            mb_sb = singles.tile([P, NIC], F32)
            nc.sync.dma_start(mb_sb, mb.rearrange("(c p) -> p c", p=P))

            # ---- style projection: s[ic, b] = mod_w @ style.T + mod_b -----
            s_sb = singles.tile([P, NIC, BPC], F32)
            s2_sb = singles.tile([P, NIC, BPC], F32)
            for c in range(NIC):
                ps = psum.tile([P, NFREE], F32, tag="ps")
                for k in range(SD // P):
                    nc.tensor.matmul(
                        ps[:, :BPC],
                        mwT_sb[:, k, c * P : (c + 1) * P],
                        stT_sb[:, k, :],
                        start=(k == 0),
                        stop=(k == SD // P - 1),
                    )
                nc.vector.tensor_scalar_add(s_sb[:, c, :], ps[:, :BPC], mb_sb[:, c : c + 1])
                nc.vector.tensor_mul(s2_sb[:, c, :], s_sb[:, c, :], s_sb[:, c, :])

            # ---- weights + x DMAs, interleaved for startup latency --------
            # bw c0 first (U12 c0 feeds conv MM #1..12), then x bands 0..2
            # (V band 0), then remaining bw chunks ahead of x bands 3..7.
            bw = singles.tile([P, NIC, KK, OC], BF16)
            u12 = singles.tile([P, NIC, 2, KS, OC], BF16)
            ws = singles.tile([P, NIC, OC], F32)

            # E/O parity planes: E row e = image row 2e-1 (e=0 pad),
            # O row o = image row 2o (o=32 pad); data cols 1..64.
            eo = singles.tile([P, 2, NIC, EROWS, PWX], BF16)

            def u12_emit(c):
                for kx in range(KS):
                    e = up.tile([P, OC], BF16, tag="ue")
                    nc.vector.tensor_add(e, bw[:, c, kx], bw[:, c, 6 + kx])
                    nc.vector.tensor_add(u12[:, c, 0, kx], e, bw[:, c, 3 + kx])
                    nc.vector.tensor_sub(u12[:, c, 1, kx], e, bw[:, c, 3 + kx])

            def squares_emit(c):
                nc.scalar.square(ws[:, c, :], bw[:, c, 0, :])
                for k in range(1, KK):
                    sq = sqp.tile([P, OC], F32, tag="sq")
                    nc.scalar.square(sq, bw[:, c, k, :])
                    nc.vector.tensor_add(ws[:, c, :], ws[:, c, :], sq)

            def pad_memsets():
                for par in range(2):
                    for c in range(NIC):
                        v = eo[:, par, c]
                        nc.gpsimd.memset(v[:, :, 0:1], 0.0)
                        nc.gpsimd.memset(v[:, :, 1 + W : PWX], 0.0)
                for c in range(NIC):
                    nc.gpsimd.memset(eo[:, 0, c, 0, :], 0.0)
                    nc.gpsimd.memset(eo[:, 1, c, EROWS - 1, :], 0.0)

            # x band j (image rows 8j..8j+7): even rows -> O[4j..4j+4),
            # odd rows -> E[4j+1..4j+5); scale by s[ic,b], cast to bf16.
            def xprep_band(b, j):
                for c in range(NIC):
                    xst = xstage.tile([P, RB, W], F32, tag="xst")
                    nc.sync.dma_start(
                        xst, xs[b, c * P : (c + 1) * P, j * RB : (j + 1) * RB, :]
                    )
                    hb = RB // 2
                    nc.gpsimd.tensor_scalar_mul(
                        eo[:, 1, c, 4 * j : 4 * j + hb, 1 : 1 + W],
                        xst[:, 0:RB:2, :],
                        s_sb[:, c, b : b + 1],
                    )
                    nc.gpsimd.tensor_scalar_mul(
                        eo[:, 0, c, 4 * j + 1 : 4 * j + 1 + hb, 1 : 1 + W],
                        xst[:, 1:RB:2, :],
                        s_sb[:, c, b : b + 1],
                    )

            # ---- V = B^T d : flat contiguous ops on the parity planes -----
            vt = singles.tile([P, VR, NIC, 4, TB, PWX], BF16)

            def vprep(g):
                r = g % VR
                for c in range(NIC):
                    E0 = eo[:, 0, c, 8 * g : 8 * g + TB, :]
                    E1 = eo[:, 0, c, 8 * g + 1 : 8 * g + 1 + TB, :]
                    O0 = eo[:, 1, c, 8 * g : 8 * g + TB, :]
                    O1 = eo[:, 1, c, 8 * g + 1 : 8 * g + 1 + TB, :]
                    nc.vector.tensor_sub(vt[:, r, c, 0], E0, E1)
                    nc.vector.tensor_add(vt[:, r, c, 1], O0, E1)
                    nc.vector.tensor_sub(vt[:, r, c, 2], E1, O0)
                    nc.vector.tensor_sub(vt[:, r, c, 3], O0, O1)

            # ---- demod scalars: d, .5d, -.5d, -d --------------------------
            dsq = singles.tile([P, NOC, BPC], F32)
            dm_sb = singles.tile([P, NOC, BPC], F32)
            dmh_sb = singles.tile([P, NOC, BPC], F32)
            ndmh_sb = singles.tile([P, NOC, BPC], F32)
            ndm_sb = singles.tile([P, NOC, BPC], F32)
            eps_sb = singles.tile([P, 1], F32)
            nc.vector.memset(eps_sb, EPS_FOLDED)

            def demod_emit():
                for o in range(NOC):
                    pd = psum.tile([P, NFREE], F32, tag="ps")
                    for c in range(NIC):
                        nc.tensor.matmul(
                            pd[:, :BPC],
                            ws[:, c, o * P : (o + 1) * P],
                            s2_sb[:, c, :],
                            start=(c == 0),
                            stop=(c == NIC - 1),
                        )
                    nc.scalar.activation(
                        out=dsq[:, o, :],
                        in_=pd[:, :BPC],
                        func=mybir.ActivationFunctionType.Sqrt,
                        bias=eps_sb[:],
                        scale=1.0,
                    )
                    nc.vector.reciprocal(out=dm_sb[:, o, :], in_=dsq[:, o, :])
                    nc.vector.tensor_scalar_mul(dmh_sb[:, o, :], dm_sb[:, o, :], 0.5)
                    nc.vector.tensor_scalar_mul(ndmh_sb[:, o, :], dm_sb[:, o, :], -0.5)
                    nc.vector.tensor_scalar_mul(ndm_sb[:, o, :], dm_sb[:, o, :], -1.0)

            # ---- conv band: 4 M_k planes, k-outer contiguous PSUM groups --
            def conv(b, g):
                r = g % VR
                for o in range(NOC):
                    osl = slice(o * P, (o + 1) * P)
                    pk = [
                        psum.tile([P, NFREE], F32, tag="ps", name=f"pk{i}")
                        for i in range(4)
                    ]
                    uof = [
                        lambda c, kx: bw[:, c, kx, osl],
                        lambda c, kx: u12[:, c, 0, kx, osl],
                        lambda c, kx: u12[:, c, 1, kx, osl],
                        lambda c, kx: bw[:, c, 6 + kx, osl],
                    ]
                    for k in range(4):
                        for c in range(NIC):
                            for kx in range(KS):
                                nc.tensor.matmul(
                                    pk[k],
                                    uof[k](c, kx),
                                    vt[:, r, c, k, :, kx : kx + W],
                                    start=(c == 0 and kx == 0),
                                    stop=(c == NIC - 1 and kx == KS - 1),
                                )
                    dm = dm_sb[:, o, b : b + 1]
                    dmh = dmh_sb[:, o, b : b + 1]
                    ndmh = ndmh_sb[:, o, b : b + 1]
                    ndm = ndm_sb[:, o, b : b + 1]
                    a1 = otmp.tile([P, TB, W], F32, tag="a1")
                    t2 = otmp.tile([P, TB, W], F32, tag="t2")
                    t3 = otmp.tile([P, TB, W], F32, tag="t2")
                    ot = outp.tile([P, 2 * TB, W], F32, tag="ot")
                    p0 = pk[0].rearrange("p (r w) -> p r w", w=W)
                    p1 = pk[1].rearrange("p (r w) -> p r w", w=W)
                    p2 = pk[2].rearrange("p (r w) -> p r w", w=W)
                    p3 = pk[3].rearrange("p (r w) -> p r w", w=W)
                    # a1 = .5d*M1' on ACT (PSUM-read + per-partition scale)
                    nc.scalar.activation(
                        out=a1, in_=p1,
                        func=mybir.ActivationFunctionType.Copy,
                        scale=dmh,
                    )
                    nc.vector.scalar_tensor_tensor(
                        t2, p0, dm, a1, op0=_ALU.mult, op1=_ALU.add
                    )
                    nc.vector.scalar_tensor_tensor(
                        ot[:, 0 : 2 * TB : 2, :], p2, dmh, t2,
                        op0=_ALU.mult, op1=_ALU.add,
                    )
                    nc.vector.scalar_tensor_tensor(
                        t3, p2, ndmh, a1, op0=_ALU.mult, op1=_ALU.add
                    )
                    nc.vector.scalar_tensor_tensor(
                        ot[:, 1 : 2 * TB : 2, :], p3, ndm, t3,
                        op0=_ALU.mult, op1=_ALU.add,
                    )
                    nc.sync.dma_start(
                        out[b, osl, g * 2 * TB : (g + 1) * 2 * TB, :], ot
                    )

            # ---- emission schedule ----------------------------------------
            nc.sync.dma_start(bw[:, 0], wTb[0:P])
            u12_emit(0)
            squares_emit(0)
            pad_memsets()
            for j in range(3):
                xprep_band(0, j)
            vprep(0)
            for c in range(1, NIC):
                nc.sync.dma_start(bw[:, c], wTb[c * P : (c + 1) * P])
                u12_emit(c)
                squares_emit(c)
            with tc.high_priority(offset=-100000):
                demod_emit()
            conv(0, 0)
            for j in range(3, NXB):
                xprep_band(0, j)
            for g in range(1, NBK):
                vprep(g)
                conv(0, g)
            for j in range(NXB):
                xprep_band(1, j)
            for g in range(NBK):
                vprep(g)
                conv(1, g)

    return nc


_NC = None


def _get_nc():
    global _NC
    if _NC is None:
        _NC = build_nc()
    return _NC


def prep_in_maps(x, style, weight, mod_w, mod_b):
    x = np.ascontiguousarray(x, dtype=np.float32)
    style = np.asarray(style, dtype=np.float32)
    weight = np.asarray(weight, dtype=np.float32)
    mod_w = np.asarray(mod_w, dtype=np.float32)
    mod_b = np.ascontiguousarray(mod_b, dtype=np.float32)

    # host-side layout prep (replicated params); bf16 cast is bit-identical
    # to the on-device DVE cast (round-to-nearest-even)
    wTb = np.ascontiguousarray(
        weight[0].transpose(1, 2, 3, 0).reshape(IC, KK, OC).astype(ml_dtypes.bfloat16)
    )
    mwT = np.ascontiguousarray(mod_w.T)

    in_maps = []
    for i in range(NCORES):
        sl = slice(i * BPC, (i + 1) * BPC)
        in_maps.append(
            {
                "xs": np.ascontiguousarray(x[sl]),
                "stT": np.ascontiguousarray(style[sl].T),
                "wTb": wTb,
                "mwT": mwT,
                "mb": mod_b,
            }
        )
    return in_maps


def kernel(x, style, weight, mod_w, mod_b):
    in_maps = prep_in_maps(x, style, weight, mod_w, mod_b)
    nc = _get_nc()
    res = run_bass_kernel_spmd(nc, in_maps, core_ids=list(range(NCORES)))
    return np.concatenate([r["out"] for r in res.results], axis=0)


# revision 8
# speedup vs baseline: 1.6022x; 1.6022x over previous
"""Trainium2 Bass kernel for modulated (StyleGAN2-style) 3x3 conv, groups=batch.

Full-input contract: kernel(**inputs) takes the unsharded numpy inputs and
returns the full (16, 512, 64, 64) fp32 output. Internally the batch is
sharded 2-per-core across 8 NeuronCores (data parallel); weight/mod params
are replicated (weights shipped pre-transposed as bf16; the cast is
bit-identical to an on-device round-to-nearest cast).

Math (matching the jax reference):
    s      = style @ mod_w.T + mod_b                      # (B, IC)
    wmod   = SCALE * w * s[:,None,:,None,None]
    demod  = rsqrt(sum(wmod^2, (ic,kh,kw)) + 1e-8)        # (B, OC)
    out    = conv(x, wmod*demod, groups=batch)

Device decomposition per core (2 samples):
    conv(x, w*s) == conv(x*s, w)   -> fold s into the x fp32->bf16 pad/cast
    demod*SCALE  == rsqrt(ss + eps*IC*K*K) with ss = sum_ic WS[oc,ic]*s2[b,ic]
    conv: 1D Winograd F(2,3) along y, direct 3-tap along x:
      V0 = E[t]-E[t+1]  V1 = O[t]+E[t+1]  V2 = E[t+1]-O[t]  V3 = O[t]-O[t+1]
    where E/O are the even/odd rows of the zero-padded scaled image, stored
    deinterleaved so every V combo is a flat contiguous DVE op (2x mode).
      U0 = w[ky=0], U1' = w0+w1+w2, U2' = w0-w1+w2, U3 = w[ky=2]
    (A^T's 0.5 on the U1/U2 terms is folded into the demod scalars.)
      M_k[oc, t, x] = sum_{ic,kx} U_k[ic,oc] * V_k[ic, t, x+kx]   (PE, PSUM)
      out[2t]   = (M0 + .5*M1' + .5*M2') * demod
      out[2t+1] = (.5*M1' - .5*M2' - M3) * demod
    48 matmuls per (band, oc-chunk) vs 72 direct: 1.5x less PE work.
"""

import sys

for _p in ("/opt/trn_rl_repo",):
    if _p not in sys.path:
        sys.path.append(_p)

import ml_dtypes
import numpy as np

import concourse.bass as bass
import concourse.tile as tile
from concourse import mybir
from concourse.bass_utils import run_bass_kernel_spmd

# ---------------------------------------------------------------------------
# Workaround for this container's walrus build: an instruction can carry only
# one semaphore wait (two for EventSemaphore), but Tile emits up to two per
# instruction (and the exit drain gets one per logical processor), which
# walrus rejects with "Too many sync wait commands". Fix at the BIR-JSON
# level: move excess waits onto NoOp carrier instructions inserted directly
# before the offender on the same engine — semantically identical (all waits
# still satisfied before the instruction executes, per-engine order kept).
# ---------------------------------------------------------------------------
import json as _json

_SPLIT_OK_ENGINES = {"PE", "DVE", "Activation", "Pool", "SP"}
_orig_to_json_bytes = bass.Bass.to_json_bytes


def _to_json_bytes_split_waits(self):
    raw = _orig_to_json_bytes(self)
    m = _json.loads(raw)
    changed = False
    for fn in m.get("functions", []):
        for bb in fn.get("blocks", []):
            insts = bb.get("instructions", [])
            new_insts = []
            for inst in insts:
                si = inst.get("sync_info")
                waits = (si or {}).get("on_wait") or []
                op = inst.get("opcode", "")
                limit = 2 if op == "EventSemaphore" else 1
                if len(waits) > limit:
                    eng = inst.get("engine")
                    assert eng in _SPLIT_OK_ENGINES, (
                        f"instruction {inst.get('name')} on engine {eng} has "
                        f"{len(waits)} waits; carrier NoOp not known-safe there"
                    )
                    changed = True
                    keep = waits[-limit:]
                    for i, w in enumerate(waits[:-limit]):
                        new_insts.append(
                            {
                                "debug": inst.get("debug", 0),
                                "engine": eng,
                                "ins": [],
                                "name": f"{inst['name']}.w{i}",
                                "opcode": "NoOp",
                                "outs": [],
                                "sync_info": {"on_wait": [w], "on_update": []},
                            }
                        )
                    si["on_wait"] = keep
                new_insts.append(inst)
            bb["instructions"] = new_insts
    if not changed:
        return raw
    return _json.dumps(m).encode()


bass.Bass.to_json_bytes = _to_json_bytes_split_waits

# ---------------------------------------------------------------------------
# Problem constants (hardcoded per spec)
# ---------------------------------------------------------------------------
B, IC, OC, H, W, KS, SD = 16, 512, 512, 64, 64, 3, 512
NCORES = 8
BPC = B // NCORES          # samples per core
P = 128
NIC = IC // P              # 4 ic chunks
NOC = OC // P              # 4 oc chunks
KK = KS * KS               # 9
PWX = W + 4                # 68 padded row width (4B-aligned rows)
EROWS = H // 2 + 1         # 33 rows in each of the E/O parity planes
TT = H // 2                # 32 winograd row-pair tiles
TB = 8                     # tiles per band/group (16 output rows)
NBK = TT // TB             # 4 bands per sample
NFREE = TB * W             # 512 matmul free dim
VR = 2                     # V ring depth (bands in flight)
RB = 8                     # x staging rows per band
NXB = H // RB              # 8 x-staging bands
# rsqrt(SCALE^2*ss + 1e-8) * SCALE == rsqrt(ss + 1e-8*IC*K*K)
EPS_FOLDED = 1e-8 * IC * KS * KS

F32 = mybir.dt.float32
BF16 = mybir.dt.bfloat16
_ALU = mybir.AluOpType


def build_nc():
    nc = bass.Bass()
    xs = nc.dram_tensor("xs", [BPC, IC, H, W], F32, kind="ExternalInput")
    stT = nc.dram_tensor("stT", [SD, BPC], F32, kind="ExternalInput")
    wTb = nc.dram_tensor("wTb", [IC, KK, OC], BF16, kind="ExternalInput")
    mwT = nc.dram_tensor("mwT", [SD, IC], F32, kind="ExternalInput")
    mb = nc.dram_tensor("mb", [IC], F32, kind="ExternalInput")
    out = nc.dram_tensor("out", [BPC, OC, H, W], F32, kind="ExternalOutput")

    with tile.TileContext(nc) as tc:
        with (
            tc.tile_pool(name="singles", bufs=1) as singles,
            tc.tile_pool(name="sqp", bufs=3) as sqp,
            tc.tile_pool(name="up", bufs=2) as up,
            tc.tile_pool(name="xstage", bufs=4) as xstage,
            tc.tile_pool(name="otmp", bufs=4) as otmp,
            tc.tile_pool(name="outp", bufs=4) as outp,
            tc.tile_pool(name="psum", bufs=8, space="PSUM") as psum,
        ):
            # ---- constants ------------------------------------------------
            mwT_sb = singles.tile([P, SD // P, IC], F32)
            mwT_v = mwT.rearrange("(ko ki) i -> ki ko i", ki=P)
            for k in range(SD // P):
                nc.sync.dma_start(mwT_sb[:, k], mwT_v[:, k])
            stT_sb = singles.tile([P, SD // P, BPC], F32)
            nc.sync.dma_start(stT_sb, stT.rearrange("(ko ki) b -> ki ko b", ki=P))
            mb_sb = singles.tile([P, NIC], F32)
            nc.sync.dma_start(mb_sb, mb.rearrange("(c p) -> p c", p=P))

            # ---- style projection: s[ic, b] = mod_w @ style.T + mod_b -----
            s_sb = singles.tile([P, NIC, BPC], F32)
            s2_sb = singles.tile([P, NIC, BPC], F32)
            for c in range(NIC):
                ps = psum.tile([P, NFREE], F32, tag="ps")
                for k in range(SD // P):
                    nc.tensor.matmul(
                        ps[:, :BPC],
                        mwT_sb[:, k, c * P : (c + 1) * P],
                        stT_sb[:, k, :],
                        start=(k == 0),
                        stop=(k == SD // P - 1),
                    )
                nc.vector.tensor_scalar_add(s_sb[:, c, :], ps[:, :BPC], mb_sb[:, c : c + 1])
                nc.vector.tensor_mul(s2_sb[:, c, :], s_sb[:, c, :], s_sb[:, c, :])

            # ---- weights + x DMAs, interleaved for startup latency --------
            # bw c0 first (U12 c0 feeds conv MM #1..12), then x bands 0..2
            # (V band 0), then remaining bw chunks ahead of x bands 3..7.
            bw = singles.tile([P, NIC, KK, OC], BF16)
            u12 = singles.tile([P, NIC, 2, KS, OC], BF16)
            ws = singles.tile([P, NIC, OC], F32)

            # E/O parity planes: E row e = image row 2e-1 (e=0 pad),
            # O row o = image row 2o (o=32 pad); data cols 1..64.
            eo = singles.tile([P, 2, NIC, EROWS, PWX], BF16)

            def u12_emit(c):
                for kx in range(KS):
                    e = up.tile([P, OC], BF16, tag="ue")
                    nc.vector.tensor_add(e, bw[:, c, kx], bw[:, c, 6 + kx])
                    nc.vector.tensor_add(u12[:, c, 0, kx], e, bw[:, c, 3 + kx])
                    nc.vector.tensor_sub(u12[:, c, 1, kx], e, bw[:, c, 3 + kx])

            def squares_emit(c):
                nc.scalar.square(ws[:, c, :], bw[:, c, 0, :])
                for k in range(1, KK):
                    sq = sqp.tile([P, OC], F32, tag="sq")
                    nc.scalar.square(sq, bw[:, c, k, :])
                    nc.vector.tensor_add(ws[:, c, :], ws[:, c, :], sq)

            def pad_memsets():
                for par in range(2):
                    for c in range(NIC):
                        v = eo[:, par, c]
                        nc.gpsimd.memset(v[:, :, 0:1], 0.0)
                        nc.gpsimd.memset(v[:, :, 1 + W : PWX], 0.0)
                for c in range(NIC):
                    nc.gpsimd.memset(eo[:, 0, c, 0, :], 0.0)
                    nc.gpsimd.memset(eo[:, 1, c, EROWS - 1, :], 0.0)

            # x band j (image rows 16j..16j+15), DMA on the ACT hwdge queue
            # (parallel with weights on the SP queue). Even rows ->
            # O[8j..8j+8), odd rows -> E[8j+1..8j+9); scale, cast to bf16.
            XB = 2 * RB  # 16-row x bands

            def xband_dma(b, j):
                tiles = []
                for c in range(NIC):
                    xst = xstage.tile([P, XB, W], F32, tag="xst", name=f"xst{b}{j}{c}")
                    nc.scalar.dma_start(
                        xst, xs[b, c * P : (c + 1) * P, j * XB : (j + 1) * XB, :]
                    )
                    tiles.append(xst)
                return tiles

            def xprep_copies(b, j, tiles):
                for c in range(NIC):
                    xst = tiles[c]
                    nc.scalar.activation(
                        out=eo[:, 1, c, 8 * j : 8 * j + RB, 1 : 1 + W],
                        in_=xst[:, 0:XB:2, :],
                        func=mybir.ActivationFunctionType.Copy,
                        scale=s_sb[:, c, b : b + 1],
                    )
                    nc.scalar.activation(
                        out=eo[:, 0, c, 8 * j + 1 : 8 * j + 1 + RB, 1 : 1 + W],
                        in_=xst[:, 1:XB:2, :],
                        func=mybir.ActivationFunctionType.Copy,
                        scale=s_sb[:, c, b : b + 1],
                    )

            # ---- V = B^T d : flat contiguous ops on the parity planes -----
            vt = singles.tile([P, VR, NIC, 4, TB, PWX], BF16)

            def vprep(g):
                r = g % VR
                for c in range(NIC):
                    E0 = eo[:, 0, c, 8 * g : 8 * g + TB, :]
                    E1 = eo[:, 0, c, 8 * g + 1 : 8 * g + 1 + TB, :]
                    O0 = eo[:, 1, c, 8 * g : 8 * g + TB, :]
                    O1 = eo[:, 1, c, 8 * g + 1 : 8 * g + 1 + TB, :]
                    nc.vector.tensor_sub(vt[:, r, c, 0], E0, E1)
                    nc.vector.tensor_add(vt[:, r, c, 1], O0, E1)
                    nc.vector.tensor_sub(vt[:, r, c, 2], E1, O0)
                    nc.vector.tensor_sub(vt[:, r, c, 3], O0, O1)

            # ---- demod scalars: d, .5d, -.5d, -d --------------------------
            dsq = singles.tile([P, NOC, BPC], F32)
            dm_sb = singles.tile([P, NOC, BPC], F32)
            dmh_sb = singles.tile([P, NOC, BPC], F32)
            ndmh_sb = singles.tile([P, NOC, BPC], F32)
            ndm_sb = singles.tile([P, NOC, BPC], F32)
            eps_sb = singles.tile([P, 1], F32)
            nc.vector.memset(eps_sb, EPS_FOLDED)

            def demod_emit():
                for o in range(NOC):
                    pd = psum.tile([P, NFREE], F32, tag="ps")
                    for c in range(NIC):
                        nc.tensor.matmul(
                            pd[:, :BPC],
                            ws[:, c, o * P : (o + 1) * P],
                            s2_sb[:, c, :],
                            start=(c == 0),
                            stop=(c == NIC - 1),
                        )
                    nc.scalar.activation(
                        out=dsq[:, o, :],
                        in_=pd[:, :BPC],
                        func=mybir.ActivationFunctionType.Sqrt,
                        bias=eps_sb[:],
                        scale=1.0,
                    )
                    nc.vector.reciprocal(out=dm_sb[:, o, :], in_=dsq[:, o, :])
                    nc.vector.tensor_scalar_mul(dmh_sb[:, o, :], dm_sb[:, o, :], 0.5)
                    nc.vector.tensor_scalar_mul(ndmh_sb[:, o, :], dm_sb[:, o, :], -0.5)
                    nc.vector.tensor_scalar_mul(ndm_sb[:, o, :], dm_sb[:, o, :], -1.0)

            # ---- conv band: 4 M_k planes, k-outer contiguous PSUM groups --
            def conv(b, g):
                r = g % VR
                for o in range(NOC):
                    osl = slice(o * P, (o + 1) * P)
                    pk = [
                        psum.tile([P, NFREE], F32, tag="ps", name=f"pk{i}")
                        for i in range(4)
                    ]
                    uof = [
                        lambda c, kx: bw[:, c, kx, osl],
                        lambda c, kx: u12[:, c, 0, kx, osl],
                        lambda c, kx: u12[:, c, 1, kx, osl],
                        lambda c, kx: bw[:, c, 6 + kx, osl],
                    ]
                    for k in range(4):
                        for c in range(NIC):
                            for kx in range(KS):
                                nc.tensor.matmul(
                                    pk[k],
                                    uof[k](c, kx),
                                    vt[:, r, c, k, :, kx : kx + W],
                                    start=(c == 0 and kx == 0),
                                    stop=(c == NIC - 1 and kx == KS - 1),
                                )
                    dm = dm_sb[:, o, b : b + 1]
                    dmh = dmh_sb[:, o, b : b + 1]
                    ndmh = ndmh_sb[:, o, b : b + 1]
                    ndm = ndm_sb[:, o, b : b + 1]
                    a1 = otmp.tile([P, TB, W], F32, tag="a1")
                    t2 = otmp.tile([P, TB, W], F32, tag="t2")
                    t3 = otmp.tile([P, TB, W], F32, tag="t2")
                    ot = outp.tile([P, 2 * TB, W], F32, tag="ot")
                    p0 = pk[0].rearrange("p (r w) -> p r w", w=W)
                    p1 = pk[1].rearrange("p (r w) -> p r w", w=W)
                    p2 = pk[2].rearrange("p (r w) -> p r w", w=W)
                    p3 = pk[3].rearrange("p (r w) -> p r w", w=W)
                    # a1 = .5d*M1' on ACT (PSUM-read + per-partition scale)
                    nc.scalar.activation(
                        out=a1, in_=p1,
                        func=mybir.ActivationFunctionType.Copy,
                        scale=dmh,
                    )
                    nc.vector.scalar_tensor_tensor(
                        t2, p0, dm, a1, op0=_ALU.mult, op1=_ALU.add
                    )
                    nc.vector.scalar_tensor_tensor(
                        ot[:, 0 : 2 * TB : 2, :], p2, dmh, t2,
                        op0=_ALU.mult, op1=_ALU.add,
                    )
                    nc.vector.scalar_tensor_tensor(
                        t3, p2, ndmh, a1, op0=_ALU.mult, op1=_ALU.add
                    )
                    nc.vector.scalar_tensor_tensor(
                        ot[:, 1 : 2 * TB : 2, :], p3, ndm, t3,
                        op0=_ALU.mult, op1=_ALU.add,
                    )
                    nc.sync.dma_start(
                        out[b, osl, g * 2 * TB : (g + 1) * 2 * TB, :], ot
                    )

            # ---- emission schedule ----------------------------------------
            NJB = H // XB  # 4 x bands per sample
            # x DMAs for bands 0,1 first on the ACT queue; weights on SP.
            t00 = xband_dma(0, 0)
            t01 = xband_dma(0, 1)
            for c in range(NIC):
                nc.sync.dma_start(bw[:, c], wTb[c * P : (c + 1) * P])
            pad_memsets()
            xprep_copies(0, 0, t00)
            for c in range(NIC):
                u12_emit(c)
            xprep_copies(0, 1, t01)
            vprep(0)
            # demod path: squares (ACT) + adds (DVE) after band-0 prep
            for c in range(NIC):
                squares_emit(c)
            with tc.high_priority(offset=-100000):
                demod_emit()
            conv(0, 0)
            t02 = xband_dma(0, 2)
            xprep_copies(0, 2, t02)
            vprep(1)
            conv(0, 1)
            t03 = xband_dma(0, 3)
            xprep_copies(0, 3, t03)
            vprep(2)
            conv(0, 2)
            vprep(3)
            conv(0, 3)
            tj = xband_dma(1, 0)
            xprep_copies(1, 0, tj)
            for g in range(NBK):
                if g + 1 < NJB:
                    tj = xband_dma(1, g + 1)
                    xprep_copies(1, g + 1, tj)
                vprep(g)
                conv(1, g)

    return nc


_NC = None


def _get_nc():
    global _NC
    if _NC is None:
        _NC = build_nc()
    return _NC


def prep_in_maps(x, style, weight, mod_w, mod_b):
    x = np.ascontiguousarray(x, dtype=np.float32)
    style = np.asarray(style, dtype=np.float32)
    weight = np.asarray(weight, dtype=np.float32)
    mod_w = np.asarray(mod_w, dtype=np.float32)
    mod_b = np.ascontiguousarray(mod_b, dtype=np.float32)

    # host-side layout prep (replicated params); bf16 cast is bit-identical
    # to the on-device DVE cast (round-to-nearest-even)
    wTb = np.ascontiguousarray(
        weight[0].transpose(1, 2, 3, 0).reshape(IC, KK, OC).astype(ml_dtypes.bfloat16)
    )
    mwT = np.ascontiguousarray(mod_w.T)

    in_maps = []
    for i in range(NCORES):
        sl = slice(i * BPC, (i + 1) * BPC)
        in_maps.append(
            {
                "xs": np.ascontiguousarray(x[sl]),
                "stT": np.ascontiguousarray(style[sl].T),
                "wTb": wTb,
                "mwT": mwT,
                "mb": mod_b,
            }
        )
    return in_maps


def kernel(x, style, weight, mod_w, mod_b):
    in_maps = prep_in_maps(x, style, weight, mod_w, mod_b)
    nc = _get_nc()
    res = run_bass_kernel_spmd(nc, in_maps, core_ids=list(range(NCORES)))
    return np.concatenate([r["out"] for r in res.results], axis=0)
